# revision 4
# baseline (speedup 1.0000x reference)
"""GPTQ 4-bit quantized linear: out = x @ dequant(qweight, qzeros, scales, g_idx) + bias.

Full shapes: x [8192, 4096] fp16, qweight [512, 4096] int32 (8x 4-bit packed
along K), qzeros [32, 512] int32, scales [32, 4096] fp16, g_idx [4096] int32
(k // 128), bias [4096] fp16.  Output [8192, 4096] fp16.

Strategy: 2 (M) x 4 (N) grid over 8 NeuronCores.  Per core: M=4096, N=1024,
K=4096.  The matmul runs in fp8-e4m3 DoubleRow perf mode (2 k-rows/cycle,
157 TF/s) with an error-corrected 3-term decomposition:

    out = x8@W8 + xlo8@W8 + x8@Wlo8        (all three accumulate in PSUM)

where x8 = e4m3(64*x), xlo8 = e4m3(64*x - x8), W8 = e4m3(64*W),
Wlo8 = e4m3(64*W - W8).  The global 2^6 scaling keeps the hi AND lo parts
in e4m3's normal range (min normal 2^-6), so each residual quantizes at
~2.6% relative instead of hitting subnormal absolute error; measured end
rel-err ~1.3e-3 vs the 2e-2 gate.  PE work = 1.5 fp8 matmuls = 0.75x the
fp16-matmul cycle count.  The 2^-12 un-scaling rides the ACT drain copy.

Host does dequant + quantize + layout (numpy); device does only DoubleRow
matmuls and the bias/scale drain.  W (hi+lo, 8.4MB fp8) stays resident in
SBUF; x streams once per 512-wide m-block (no reload per n-block).
"""

import os
import sys

import numpy as np
import ml_dtypes

for _p in ("/opt/trn_rl_repo",):
    if _p not in sys.path and os.path.isdir(_p):
        sys.path.insert(0, _p)

import concourse.bass as bass
import concourse.mybir as mybir
import concourse.tile as tile
from concourse import bacc
from concourse.bass_utils import run_bass_kernel_spmd

dt = mybir.dt
F8 = ml_dtypes.float8_e4m3

P = 128          # partitions
JP = 8           # 4-bit values per int32
GROUP = 128      # quant group size
NPS = 512        # psum free width
AQ = 4           # a-quads (DMA granularity along k)
APQ = 4          # a's per quad
A = AQ * APQ     # 16 a-chunks of 256 k each
SCALE_BITS = 6   # global 2^6 operand scaling
DRAIN_SCALE = 1.0 / float(1 << (2 * SCALE_BITS))


def build_program(K, M, N):
    """One-core SPMD program: out[M,N] = sum over k of x-parts @ W-parts with
    fp8 DoubleRow matmuls.  Inputs are pre-quantized/packed fp8 tiles."""
    assert K == 256 * A
    MB = M // NPS        # 512-wide m superblocks
    NB = N // NPS        # psum column blocks
    assert M % NPS == 0 and N % NPS == 0

    nc = bacc.Bacc("TRN2", target_bir_lowering=False)

    # k = 1024*aq + 256*ap + 128*i + p;  tiles store (ap, i) merged as dim of 8
    xh = nc.dram_tensor("xh", [AQ, MB, P, 2 * APQ, NPS], dt.float8e4, kind="ExternalInput")
    xl = nc.dram_tensor("xl", [AQ, MB, P, 2 * APQ, NPS], dt.float8e4, kind="ExternalInput")
    wh = nc.dram_tensor("wh", [AQ, P, 2 * APQ, N], dt.float8e4, kind="ExternalInput")
    wl = nc.dram_tensor("wl", [AQ, P, 2 * APQ, N], dt.float8e4, kind="ExternalInput")
    bs = nc.dram_tensor("bs", [P, N], dt.float16, kind="ExternalInput")
    out = nc.dram_tensor("out", [M, N], dt.float16, kind="ExternalOutput")

    DR = mybir.MatmulPerfMode.DoubleRow

    from contextlib import ExitStack

    with tile.TileContext(nc) as tc, ExitStack() as ctx:
        const = ctx.enter_context(tc.tile_pool(name="const", bufs=1))
        # wpool: 8 persistent untagged tiles (1 slot each)
        wpool = ctx.enter_context(tc.tile_pool(name="wpool", bufs=1))
        # xpool: tags xh/xl cycle 8 slots each (4 aq live + 4 prefetch)
        xpool = ctx.enter_context(tc.tile_pool(name="xpool", bufs=2 * AQ))
        opool = ctx.enter_context(tc.tile_pool(name="opool", bufs=4))
        psum = ctx.enter_context(tc.tile_pool(name="psum", bufs=8, space="PSUM"))

        # PE warmup: dummy fp16 matmuls with no DMA dependency so the HAM
        # clock-gate opens (1.2 -> 2.4 GHz) before the first real matmul.
        warm_src = const.tile([P, NPS], dt.float16)
        nc.gpsimd.memset(warm_src[:], 0.0)
        warm_ps = psum.tile([P, NPS], dt.float32, tag="ps")
        NWARM = 12
        for wi in range(NWARM):
            nc.tensor.matmul(
                warm_ps[:], warm_src[:, :P], warm_src[:],
                start=(wi == 0), stop=(wi == NWARM - 1),
            )

        bias_t = const.tile([P, N], dt.float16)
        nc.sync.dma_start(bias_t[:], bs[:])

        # resident W tiles: [128, 8, N] fp8 per (aq, hi/lo)
        W8T, WLT = [], []
        for aq in range(AQ):
            t = wpool.tile([P, 2 * APQ, N], dt.float8e4, name=f"w8_{aq}")
            nc.gpsimd.dma_start(t[:], wh[aq])
            W8T.append(t)
            t = wpool.tile([P, 2 * APQ, N], dt.float8e4, name=f"wl_{aq}")
            nc.gpsimd.dma_start(t[:], wl[aq])
            WLT.append(t)

        for mb in range(MB):
            # stream this m-block's x tiles: [128, 8, 512] fp8 per (aq, hi/lo)
            XHT, XLT = [], []
            for aq in range(AQ):
                t = xpool.tile([P, 2 * APQ, NPS], dt.float8e4, tag="xh", name=f"xh_t_{aq}")
                nc.sync.dma_start(t[:], xh[aq, mb])
                XHT.append(t)
                t = xpool.tile([P, 2 * APQ, NPS], dt.float8e4, tag="xl", name=f"xl_t_{aq}")
                nc.sync.dma_start(t[:], xl[aq, mb])
                XLT.append(t)

            for ms in range(NPS // P):
                msl = slice(ms * P, (ms + 1) * P)
                pss = [psum.tile([P, NPS], dt.float32, tag="ps", name=f"ps_{nb}")
                       for nb in range(NB)]
                for aq in range(AQ):
                    for ap in range(APQ):
                        isl = slice(2 * ap, 2 * ap + 2)
                        first = (aq == 0 and ap == 0)
                        last = (aq == AQ - 1 and ap == APQ - 1)
                        lh = XHT[aq][:, isl, msl]
                        ll = XLT[aq][:, isl, msl]
                        for nb in range(NB):
                            nsl = slice(nb * NPS, (nb + 1) * NPS)
                            nc.tensor.matmul(
                                pss[nb][:], lh, W8T[aq][:, isl, nsl],
                                start=first, stop=False, perf_mode=DR,
                            )
                            nc.tensor.matmul(
                                pss[nb][:], lh, WLT[aq][:, isl, nsl],
                                start=False, stop=False, perf_mode=DR,
                            )
                        for nb in range(NB):
                            nsl = slice(nb * NPS, (nb + 1) * NPS)
                            nc.tensor.matmul(
                                pss[nb][:], ll, W8T[aq][:, isl, nsl],
                                start=False, stop=last, perf_mode=DR,
                            )
                for nb in range(NB):
                    nsl = slice(nb * NPS, (nb + 1) * NPS)
                    oc = opool.tile([P, NPS], dt.float16, tag="oc")
                    nc.scalar.activation(
                        oc[:], pss[nb][:],
                        mybir.ActivationFunctionType.Copy, scale=DRAIN_SCALE,
                    )
                    ob = opool.tile([P, NPS], dt.float16, tag="ob")
                    nc.vector.tensor_tensor(
                        ob[:], oc[:], bias_t[:, nsl], op=mybir.AluOpType.add
                    )
                    nc.gpsimd.dma_start(
                        out[mb * NPS + ms * P: mb * NPS + (ms + 1) * P, nsl],
                        ob[:],
                    )
    nc.finalize()
    return nc


def _pack_x(x8_bytes, m_split):
    """x8_bytes: [M_full, K] f8.  Returns per-m-row-shard arrays
    [AQ, MBc, 128, 8, 512] with k = 1024*aq + 256*ap + 128*i + p."""
    M_full, K = x8_bytes.shape
    xT = np.ascontiguousarray(x8_bytes.T)                # [K, M_full]
    MBf = M_full // NPS
    a = xT.reshape(AQ, APQ, 2, P, MBf, NPS)              # (aq, ap, i, p, mbf, m)
    a = np.ascontiguousarray(a.transpose(0, 4, 3, 1, 2, 5))  # (aq, mbf, p, ap, i, m)
    a = a.reshape(AQ, MBf, P, 2 * APQ, NPS)
    MBc = MBf // m_split
    return [np.ascontiguousarray(a[:, mi * MBc:(mi + 1) * MBc]) for mi in range(m_split)]


def _pack_w(w_bytes, n_split):
    """w_bytes: [K, N_full] f8.  Returns per-n-col-shard arrays
    [AQ, 128, 8, Nc]."""
    K, N_full = w_bytes.shape
    Nc = N_full // n_split
    outs = []
    for ni in range(n_split):
        wc = w_bytes[:, ni * Nc:(ni + 1) * Nc]
        a = wc.reshape(AQ, APQ, 2, P, Nc)                # (aq, ap, i, p, n)
        a = np.ascontiguousarray(a.transpose(0, 3, 1, 2, 4))  # (aq, p, ap, i, n)
        outs.append(a.reshape(AQ, P, 2 * APQ, Nc))
    return outs


def host_prep(x, qweight, qzeros, scales, g_idx, bias, m_split, n_split):
    """Dequantize W, split x/W into scaled e4m3 hi+lo parts, pack per core."""
    M_full, K = x.shape
    G, N_full = scales.shape

    shifts = (np.arange(JP, dtype=np.int32) * 4)
    w = ((qweight[:, None, :] >> shifts[None, :, None]) & 15).reshape(K, N_full)
    z = ((qzeros[:, :, None] >> shifts[None, None, :]) & 15).reshape(G, N_full) + 1
    cg = np.asarray(g_idx[::GROUP])
    assert np.array_equal(np.repeat(cg, GROUP), np.asarray(g_idx)), \
        "g_idx must be uniform within 128-wide k chunks"
    iw = (w - z[cg].repeat(GROUP, axis=0)).astype(np.float32)
    W64 = iw * (np.asarray(scales, np.float32)[cg].repeat(GROUP, axis=0) * 64.0)

    W8 = W64.astype(np.float16).astype(F8)
    Wlo = (W64 - W8.astype(np.float32)).astype(np.float16)
    Wlo8 = Wlo.astype(F8)

    xs = np.asarray(x) * np.float16(64.0)
    x8 = xs.astype(F8)
    dx = xs - x8.astype(np.float16)
    xlo8 = dx.astype(F8)

    xh_shards = _pack_x(x8, m_split)
    xl_shards = _pack_x(xlo8, m_split)
    wh_shards = _pack_w(W8, n_split)
    wl_shards = _pack_w(Wlo8, n_split)

    Nc = N_full // n_split
    bias = np.asarray(bias)
    in_maps = []
    for mi in range(m_split):
        for ni in range(n_split):
            in_maps.append({
                "xh": xh_shards[mi],
                "xl": xl_shards[mi],
                "wh": wh_shards[ni],
                "wl": wl_shards[ni],
                "bs": np.ascontiguousarray(
                    np.broadcast_to(bias[ni * Nc:(ni + 1) * Nc], (P, Nc))
                ),
            })
    return in_maps, M_full // m_split, Nc


_PROGRAM_CACHE = {}


def _get_program(K, M, N):
    key = (K, M, N)
    if key not in _PROGRAM_CACHE:
        _PROGRAM_CACHE[key] = build_program(K, M, N)
    return _PROGRAM_CACHE[key]


def kernel(x, qweight, qzeros, scales, g_idx, bias, trace=False, trace_kwargs=None):
    m_split, n_split = 2, 4
    x = np.asarray(x)
    qweight = np.asarray(qweight)
    qzeros = np.asarray(qzeros)
    scales = np.asarray(scales)
    g_idx = np.asarray(g_idx)
    bias = np.asarray(bias)
    M_full, K = x.shape
    N_full = scales.shape[1]
    in_maps, M, N = host_prep(x, qweight, qzeros, scales, g_idx, bias,
                              m_split, n_split)
    nc = _get_program(K, M, N)
    kw = {}
    if trace:
        kw = dict(trace=True, **(trace_kwargs or {}))
    rb = run_bass_kernel_spmd(nc, in_maps, list(range(m_split * n_split)), **kw)
    out = np.empty((M_full, N_full), dtype=np.float16)
    ci = 0
    for mi in range(m_split):
        for ni in range(n_split):
            out[mi * M:(mi + 1) * M, ni * N:(ni + 1) * N] = rb.results[ci]["out"]
            ci += 1
    kernel.last_results = rb
    return out


# revision 7
# speedup vs baseline: 1.8417x; 1.8417x over previous
"""GPTQ 4-bit quantized linear: out = x @ dequant(qweight, qzeros, scales, g_idx) + bias.

Full shapes: x [8192, 4096] fp16, qweight [512, 4096] int32 (8x 4-bit packed
along K), qzeros [32, 512] int32, scales [32, 4096] fp16, g_idx [4096] int32
(k // 128), bias [4096] fp16.  Output [8192, 4096] fp16.

Strategy: 2 (M) x 4 (N) grid over 8 NeuronCores.  Per core: M=4096, N=1024,
K=4096, all in fp16 on the PE -- but via one level of Strassen, which cuts
PE cycles to 7/8 of the direct matmul (the PE at 78.6 TF/s fp16 is the
bottleneck; fp8 DoubleRow was measured to give 2x FLOPs per cycle but the
3-term error-corrected decomposition it needs costs 3x FLOPs, a net loss).

Per core split M=2x2048, K=2x2048, N=2x512:
  P1=(A11+A22)(B11+B22)  P2=(A21+A22)B11  P3=A11(B12-B22)  P4=A22(B21-B11)
  P5=(A11+A12)B22        P6=(A21-A11)(B11+B12)  P7=(A12-A22)(B21+B22)
  C11=P1+P4-P5+P7  C12=P3+P5  C21=P2+P4  C22=P1-P2+P3+P6

The host dequantizes W and precomputes the 7 fp16 A- and B-combos (adds of
quadrants; psum stays fp32 so the fp16 combo rounding is benign -- measured
rel err ~9e-4).  The device, per 128-row m'-block, accumulates the seven
P-products in seven PSUM banks (16 matmuls of [128k,128m]x[128k,512n] each)
and combines them with 12 DVE ops ordered so PSUM banks free early.  B-combos
(14.7MB fp16) stay resident in SBUF; A-combos stream per block on SyncE.
"""

import os
import sys

import numpy as np

for _p in ("/opt/trn_rl_repo",):
    if _p not in sys.path and os.path.isdir(_p):
        sys.path.insert(0, _p)

import concourse.bass as bass
import concourse.mybir as mybir
import concourse.tile as tile
from concourse import bacc
from concourse.bass_utils import run_bass_kernel_spmd

dt = mybir.dt

P = 128          # partitions
JP = 8           # 4-bit values per int32
GROUP = 128      # quant group size
NPS = 512        # psum free width / n'-quadrant width
NPROD = 7


def build_program(K, M, N):
    """One-core SPMD program: Strassen 1-level over [M=4096,K=4096,N=1024]."""
    KH, MH, NH = K // 2, M // 2, N // 2
    KT = KH // P         # 16 k'-tiles per product
    MB = MH // P         # 16 m'-blocks
    assert NH == NPS

    nc = bacc.Bacc("TRN2", target_bir_lowering=False)

    ac = nc.dram_tensor("ac", [NPROD, MB, P, KT, P], dt.float16, kind="ExternalInput")
    bc = nc.dram_tensor("bc", [NPROD, P, KT, NPS], dt.float16, kind="ExternalInput")
    bs = nc.dram_tensor("bs", [P, N], dt.float16, kind="ExternalInput")
    out = nc.dram_tensor("out", [M, N], dt.float16, kind="ExternalOutput")

    add = mybir.AluOpType.add
    sub = mybir.AluOpType.subtract

    from contextlib import ExitStack

    with tile.TileContext(nc) as tc, ExitStack() as ctx:
        const = ctx.enter_context(tc.tile_pool(name="const", bufs=1))
        bpool = ctx.enter_context(tc.tile_pool(name="bpool", bufs=1))
        apool = ctx.enter_context(tc.tile_pool(name="apool", bufs=2))
        tpool = ctx.enter_context(tc.tile_pool(name="tpool", bufs=10))
        opool = ctx.enter_context(tc.tile_pool(name="opool", bufs=6))
        psum = ctx.enter_context(tc.tile_pool(name="psum", bufs=8, space="PSUM"))

        # PE warmup: dummy fp16 matmuls with no DMA dependency so the HAM
        # clock-gate opens (1.2 -> 2.4 GHz) before the first real matmul.
        warm_src = const.tile([P, NPS], dt.float16)
        nc.gpsimd.memset(warm_src[:], 0.0)
        warm_ps = psum.tile([P, NPS], dt.float32, tag="ps")
        NWARM = 12
        for wi in range(NWARM):
            nc.tensor.matmul(
                warm_ps[:], warm_src[:, :P], warm_src[:],
                start=(wi == 0), stop=(wi == NWARM - 1),
            )

        bias_t = const.tile([P, N], dt.float16)
        nc.sync.dma_start(bias_t[:], bs[:])

        # resident B-combo tiles: [128, 16, 512] fp16 per product (16KB lines)
        BT = []
        for prod in range(NPROD):
            t = bpool.tile([P, KT, NPS], dt.float16, name=f"b{prod}")
            nc.gpsimd.dma_start(t[:], bc[prod])
            BT.append(t)

        for mb in range(MB):
            AT = []
            for prod in range(NPROD):
                t = apool.tile([P, KT, P], dt.float16, tag=f"a{prod}",
                               name=f"a_t{prod}")
                nc.sync.dma_start(t[:], ac[prod, mb])
                AT.append(t)
            pss = []
            for prod in range(NPROD):
                ps = psum.tile([P, NPS], dt.float32, tag="ps", name=f"ps{prod}")
                for kt in range(KT):
                    nc.tensor.matmul(
                        ps[:], AT[prod][:, kt, :], BT[prod][:, kt, :],
                        start=(kt == 0), stop=(kt == KT - 1),
                    )
                pss.append(ps)
            # ACT drains each P product to SBUF fp16 (DVE can read at most
            # one PSUM operand per op); this also frees psum banks early.
            ss = []
            for prod in range(NPROD):
                s = tpool.tile([P, NPS], dt.float16, tag="s", name=f"s{prod}")
                nc.scalar.activation(
                    s[:], pss[prod][:], mybir.ActivationFunctionType.Copy
                )
                ss.append(s)
            S1, S2, S3, S4, S5, S6, S7 = ss

            def tt(name, a, b, op):
                t = tpool.tile([P, NPS], dt.float16, tag="u", name=name)
                nc.vector.tensor_tensor(t[:], a[:], b[:], op=op)
                return t

            u1 = tt("u1", S1, S4, add)
            u2 = tt("u2", S7, S5, sub)
            u3 = tt("u3", u1, u2, add)             # C11 pre-bias
            u4 = tt("u4", S3, S5, add)             # C12 pre-bias
            u5 = tt("u5", S2, S4, add)             # C21 pre-bias
            u6 = tt("u6", S1, S2, sub)
            u7 = tt("u7", S3, S6, add)
            u8 = tt("u8", u6, u7, add)             # C22 pre-bias

            m1 = mb * P
            m2 = MH + mb * P
            for name, u, nsl, mrow in (
                ("o11", u3, slice(0, NPS), m1),
                ("o12", u4, slice(NPS, N), m1),
                ("o21", u5, slice(0, NPS), m2),
                ("o22", u8, slice(NPS, N), m2),
            ):
                o = opool.tile([P, NPS], dt.float16, tag="o", name=name)
                nc.vector.tensor_tensor(o[:], u[:], bias_t[:, nsl], op=add)
                nc.gpsimd.dma_start(out[mrow:mrow + P, nsl], o[:])
    nc.finalize()
    return nc


def _pack_a(combo):
    """combo: [MH, KH] fp16 -> [MB, 128, KT, 128] with m=128*mb+mm, k=128*kt+p."""
    MH, KH = combo.shape
    a = combo.reshape(MH // P, P, KH // P, P)            # (mb, mm, kt, p)
    return np.ascontiguousarray(a.transpose(0, 3, 2, 1))  # (mb, p, kt, mm)


def _pack_b(combo):
    """combo: [KH, NPS] fp16 -> [128, KT, NPS] with k=128*kt+p."""
    KH = combo.shape[0]
    a = combo.reshape(KH // P, P, NPS)                   # (kt, p, n)
    return np.ascontiguousarray(a.transpose(1, 0, 2))    # (p, kt, n)


def host_prep(x, qweight, qzeros, scales, g_idx, bias, m_split, n_split):
    """Dequantize W, build fp16 Strassen A/B combos, pack per core."""
    M_full, K = x.shape
    G, N_full = scales.shape
    Mc = M_full // m_split
    Nc = N_full // n_split
    KH, MHc, NHc = K // 2, Mc // 2, Nc // 2

    shifts = (np.arange(JP, dtype=np.int32) * 4)
    w = ((qweight[:, None, :] >> shifts[None, :, None]) & 15).reshape(K, N_full)
    z = ((qzeros[:, :, None] >> shifts[None, None, :]) & 15).reshape(G, N_full) + 1
    cg = np.asarray(g_idx[::GROUP])
    assert np.array_equal(np.repeat(cg, GROUP), np.asarray(g_idx)), \
        "g_idx must be uniform within 128-wide k chunks"
    iw = (w - z[cg].repeat(GROUP, axis=0)).astype(np.float32)
    W16 = (iw * np.asarray(scales, np.float32)[cg].repeat(GROUP, axis=0)
           ).astype(np.float16)

    x = np.asarray(x)
    bias = np.asarray(bias)

    a_shards = []
    for mi in range(m_split):
        xm = x[mi * Mc:(mi + 1) * Mc]
        A11 = xm[:MHc, :KH]; A12 = xm[:MHc, KH:]
        A21 = xm[MHc:, :KH]; A22 = xm[MHc:, KH:]
        combos = (A11 + A22, A21 + A22, A11, A22,
                  A11 + A12, A21 - A11, A12 - A22)
        arr = np.empty((NPROD, MHc // P, P, KH // P, P), np.float16)
        for i, c in enumerate(combos):
            arr[i] = _pack_a(np.ascontiguousarray(c))
        a_shards.append(arr)

    b_shards = []
    for ni in range(n_split):
        Wc = W16[:, ni * Nc:(ni + 1) * Nc]
        B11 = Wc[:KH, :NHc]; B12 = Wc[:KH, NHc:]
        B21 = Wc[KH:, :NHc]; B22 = Wc[KH:, NHc:]
        combos = (B11 + B22, B11, B12 - B22, B21 - B11,
                  B22, B11 + B12, B21 + B22)
        arr = np.empty((NPROD, P, KH // P, NHc), np.float16)
        for i, c in enumerate(combos):
            arr[i] = _pack_b(np.ascontiguousarray(c))
        b_shards.append(arr)

    in_maps = []
    for mi in range(m_split):
        for ni in range(n_split):
            in_maps.append({
                "ac": a_shards[mi],
                "bc": b_shards[ni],
                "bs": np.ascontiguousarray(
                    np.broadcast_to(bias[ni * Nc:(ni + 1) * Nc], (P, Nc))
                ),
            })
    return in_maps, Mc, Nc


_PROGRAM_CACHE = {}


def _get_program(K, M, N):
    key = (K, M, N)
    if key not in _PROGRAM_CACHE:
        _PROGRAM_CACHE[key] = build_program(K, M, N)
    return _PROGRAM_CACHE[key]


def kernel(x, qweight, qzeros, scales, g_idx, bias, trace=False, trace_kwargs=None):
    m_split, n_split = 2, 4
    x = np.asarray(x)
    qweight = np.asarray(qweight)
    qzeros = np.asarray(qzeros)
    scales = np.asarray(scales)
    g_idx = np.asarray(g_idx)
    bias = np.asarray(bias)
    M_full, K = x.shape
    N_full = scales.shape[1]
    in_maps, M, N = host_prep(x, qweight, qzeros, scales, g_idx, bias,
                              m_split, n_split)
    nc = _get_program(K, M, N)
    kw = {}
    if trace:
        kw = dict(trace=True, **(trace_kwargs or {}))
    rb = run_bass_kernel_spmd(nc, in_maps, list(range(m_split * n_split)), **kw)
    out = np.empty((M_full, N_full), dtype=np.float16)
    ci = 0
    for mi in range(m_split):
        for ni in range(n_split):
            out[mi * M:(mi + 1) * M, ni * N:(ni + 1) * N] = rb.results[ci]["out"]
            ci += 1
    kernel.last_results = rb
    return out
